# revision 15
# baseline (speedup 1.0000x reference)
"""Trainium2 Bass kernel for LDPC-style check-node min-sum layer.

out[b, n] = prod_k sign(x[b, idx[n,k]] + eps) * min_k |x[b, idx[n,k]]|
with invalid (idx == -1) entries gathering 0, |0| treated as BIG.

Strategy (8 NeuronCores, node-sharded):
  - Host packs a node-major int32 table: bit pattern of |x[b,v]| with the
    LSB overwritten by the sign bit of x[b,v]; exact zeros become BIG with
    LSB 0.  Positive-float ordering == integer ordering, so an fp32 MIN
    over packed values yields the min magnitude (to 1 ulp) and a bitwise
    XOR yields the sign parity in the LSB.  A NEUTRAL row (BIG, LSB 0) is
    identity for both reductions, so padding slots are free.
  - dma_gather (SWDGE) indices are signed int16 (0..32767), so the table
    is laid out with three overlapping 32768-row windows, each containing
    a NEUTRAL row; every node index lands in window 1 or 2 (window 3 only
    for the two overflow nodes).  Nodes are sorted by their window-1 edge
    count so per-chunk slot caps stay tight; invalid edges are dropped.
  - Each chunk gathers its slots (256 B rows), then DVE does an fp32
    min-reduce and a pairwise xor tree per window, combines, and applies
    the parity sign with one shift + one xor.  The program is built per
    index tensor (compiled once, cached) since slot caps are data-derived.
"""

import numpy as np

CORES = 8
NODES = 65536
B = 64
K = 32
NODES_PER_CORE = NODES // CORES

NB = 2                      # node-blocks (of 128 partitions) per chunk
CH = 128 * NB               # nodes per chunk
NCHUNK = NODES_PER_CORE // CH

W = 32768                   # window size (int16-addressable rows)
NEUTRAL_V1 = 32767          # neutral view id in window 1
NEUTRAL_V2 = 32767          # neutral view id in window 2
NEUTRAL_V3 = 32765          # neutral view id in window 3
W3_BASE = 32770

BIG = np.float32(1e10)
BIG_EVEN = np.uint32(np.asarray(BIG, np.float32).view(np.uint32) & 0xFFFFFFFE)


def pack_table(x):
    """x: (B, N) float32 -> (N+2, B) int32 packed table with neutral rows.

    Layout (N = number of nodes):
      row r in [0, W-1)        : packed x[r]
      row W-1                  : NEUTRAL
      row W .. 2W-2            : packed x[W-1 .. 2W-3]
      row 2W-1                 : NEUTRAL
      row 2W, 2W+1             : packed x[2W-3+1=N-2], x[N-1]
    Window views: W1 = rows [0,W); W2 = rows [W,2W); W3 = rows [W3_BASE,.)
    """
    n = x.shape[1]
    xb = np.ascontiguousarray(x.T)                      # (N, B) f32
    pat = xb.view(np.uint32)
    mag = pat & np.uint32(0x7FFFFFFF)
    sgn = pat >> np.uint32(31)
    packed = (mag & np.uint32(0xFFFFFFFE)) | sgn
    packed = np.where(mag == 0, BIG_EVEN, packed)

    tbl = np.empty((n + 2, x.shape[0]), np.uint32)
    tbl[0:W - 1] = packed[0:W - 1]                      # x0 .. x32766
    tbl[W - 1] = BIG_EVEN                               # neutral 1
    tbl[W:2 * W - 1] = packed[W - 1:2 * W - 2]          # x32767 .. x65533
    tbl[2 * W - 1] = BIG_EVEN                           # neutral 2
    tbl[2 * W] = packed[n - 2]                          # x65534
    tbl[2 * W + 1] = packed[n - 1]                      # x65535
    return tbl.view(np.int32)


def _plan_core(ci_slice):
    """ci_slice: (npc, K) int32 raw indices (-1 invalid) for one core.

    Returns (order, chunks) where order is the node processing order and
    chunks is a list of per-chunk dicts with per-window caps and wrapped
    int16 index arrays.
    """
    npc = ci_slice.shape[0]
    e = ci_slice
    valid = e >= 0
    in1 = valid & (e <= W - 2)                          # view = v
    in2 = valid & (e >= W - 1) & (e <= 2 * W - 3)       # view = v - (W-1)
    in3 = valid & (e >= 2 * W - 2)                      # view = v - W
    k1 = in1.sum(1)

    order = np.argsort(k1, kind="stable")
    chunks = []
    for c0 in range(0, npc, CH):
        nid = order[c0:c0 + CH]
        sub = e[nid]                                    # (CH, K)
        caps = []
        views_all = []
        for m, off, neu in ((in1[nid], 0, NEUTRAL_V1),
                            (in2[nid], W - 1, NEUTRAL_V2),
                            (in3[nid], W, NEUTRAL_V3)):
            cap = int(m.sum(1).max())
            caps.append(cap)
            if cap == 0:
                views_all.append(None)
                continue
            views = np.full((CH, cap), neu, np.int32)
            rr, cc = np.nonzero(m)
            pos = (m.cumsum(1) - 1)[rr, cc]
            views[rr, pos] = sub[rr, cc] - off
            views_all.append(views)
        chunks.append({"caps": caps, "views": views_all})
    return order, chunks


def _wrap_idx(views, cap):
    """views: (CH, cap) int32 -> wrapped replicated (128, num/16) int16.

    Slot order: node q = jb*128 + p, slot t -> j = (jb*cap + t)*128 + p.
    """
    arr = views.reshape(NB, 128, cap).transpose(0, 2, 1).reshape(-1)
    wrapped = arr.reshape(-1, 16).T                     # (16, num/16)
    return np.ascontiguousarray(np.tile(wrapped, (8, 1)).astype(np.int16))


def _build_bass(plans, enable_asserts=False):
    """plans: per-core list of chunk dicts (same caps across cores enforced
    by padding to the max). Program is shared SPMD across cores, so caps
    must be identical -> caller merges caps; here plans is the merged
    [(K1, K2, K3), ...] per chunk plus total idx columns."""
    from contextlib import ExitStack

    import concourse.bass as bass
    import concourse.bacc as bacc
    import concourse.mybir as mybir
    import concourse.tile as tile

    caps_per_chunk, total_cols = plans

    nc = bacc.Bacc(
        "TRN2",
        target_bir_lowering=False,
        debug=False,
        enable_asserts=enable_asserts,
    )
    T = nc.dram_tensor("table", [NODES + 2, B], mybir.dt.int32,
                       kind="ExternalInput").ap()
    IDX = nc.dram_tensor("idx", [128, total_cols], mybir.dt.int16,
                         kind="ExternalInput").ap()
    OUT = nc.dram_tensor("out", [NODES_PER_CORE, B], mybir.dt.int32,
                         kind="ExternalOutput").ap()
    OUT_r = OUT.rearrange("(ci p j) b -> p ci j b", p=128, j=NB)

    W1 = T[0:W]
    W2 = T[W:2 * W]
    W3 = T[W3_BASE:W3_BASE + W]
    wins = (W1, W2, W3)

    with tile.TileContext(nc) as tc, ExitStack() as ctx:
        resp = ctx.enter_context(tc.tile_pool(name="resp", bufs=1))
        gp = ctx.enter_context(tc.tile_pool(name="gp", bufs=2))
        ip = ctx.enter_context(tc.tile_pool(name="ip", bufs=3))
        rp = ctx.enter_context(tc.tile_pool(name="rp", bufs=2))

        ob = resp.tile([128, NCHUNK * NB * B], mybir.dt.int32)

        col_off = 0
        for ci, caps in enumerate(caps_per_chunk):
            mins = []
            xors = []
            for wi, cap in enumerate(caps):
                if cap == 0:
                    continue
                num = CH * cap
                ncols = num // 16
                it = ip.tile([128, ncols], mybir.dt.int16, tag=f"i{wi}")
                nc.sync.dma_start(out=it[:], in_=IDX[:, col_off:col_off + ncols])
                col_off += ncols

                g = gp.tile([128, NB * cap * B], mybir.dt.int32, tag=f"g{wi}")
                nc.gpsimd.dma_gather(
                    out_ap=g[:].rearrange("p (c e) -> p c e", c=NB * cap, e=B),
                    in_ap=wins[wi],
                    idxs_ap=it[:],
                    num_idxs=num,
                    num_idxs_reg=num,
                    elem_size=B,
                    single_packet=False,
                )
                # per-partition memory: [jb, t, b]; reduce over t
                gv = g[:].rearrange("p (j t b) -> p j b t", j=NB, t=cap, b=B)
                m = rp.tile([128, NB * B], mybir.dt.float32, tag=f"m{wi}")
                nc.vector.tensor_reduce(
                    out=m[:], in_=gv.bitcast(mybir.dt.float32),
                    axis=mybir.AxisListType.X, op=mybir.AluOpType.min,
                )
                mins.append(m)

                # xor tree over t with odd-carry
                carry = None
                cur, kc = g, cap
                lvl = 0
                while kc > 1:
                    cv = cur[:].rearrange("p (j t b) -> p j t b",
                                          j=NB, t=kc, b=B)
                    if kc % 2 == 1:
                        cnew = rp.tile([128, NB * B], mybir.dt.int32,
                                       tag=f"c{wi}")
                        if carry is None:
                            nc.vector.tensor_copy(out=cnew[:],
                                                  in_=cv[:, :, kc - 1, :])
                        else:
                            nc.vector.tensor_tensor(
                                out=cnew[:], in0=carry[:],
                                in1=cv[:, :, kc - 1, :],
                                op=mybir.AluOpType.bitwise_xor)
                        carry = cnew
                        kc -= 1
                    kh = kc // 2
                    nxt = rp.tile([128, NB * kh * B], mybir.dt.int32,
                                  tag=f"x{wi}_L{lvl}")
                    lvl += 1
                    nc.vector.tensor_tensor(
                        out=nxt[:],
                        in0=cv[:, :, 0:kh, :],
                        in1=cv[:, :, kh:2 * kh, :],
                        op=mybir.AluOpType.bitwise_xor,
                    )
                    cur, kc = nxt, kh
                if carry is not None:
                    x2 = rp.tile([128, NB * B], mybir.dt.int32, tag=f"xc{wi}")
                    nc.vector.tensor_tensor(
                        out=x2[:], in0=cur[:], in1=carry[:],
                        op=mybir.AluOpType.bitwise_xor)
                    cur = x2
                xors.append(cur)

            # combine windows
            m = mins[0]
            for m2 in mins[1:]:
                mm = rp.tile([128, NB * B], mybir.dt.float32, tag="mc")
                nc.vector.tensor_tensor(out=mm[:], in0=m[:], in1=m2[:],
                                        op=mybir.AluOpType.min)
                m = mm
            s = xors[0]
            for s2 in xors[1:]:
                ss = rp.tile([128, NB * B], mybir.dt.int32, tag="sc")
                nc.vector.tensor_tensor(out=ss[:], in0=s[:], in1=s2[:],
                                        op=mybir.AluOpType.bitwise_xor)
                s = ss
            sh = rp.tile([128, NB * B], mybir.dt.int32, tag="sh")
            nc.vector.tensor_scalar(
                out=sh[:], in0=s[:], scalar1=31, scalar2=None,
                op0=mybir.AluOpType.logical_shift_left)
            nc.vector.tensor_tensor(
                out=ob[:, ci * NB * B:(ci + 1) * NB * B],
                in0=m[:].bitcast(mybir.dt.int32), in1=sh[:],
                op=mybir.AluOpType.bitwise_xor)

        nc.sync.dma_start(
            out=OUT_r,
            in_=ob[:].rearrange("p (ci j b) -> p ci j b", ci=NCHUNK, j=NB, b=B),
        )

    nc.compile()
    return nc


def _prepare(check_index_tensor):
    """Build per-core plans with caps merged across cores (SPMD program)."""
    ci = np.asarray(check_index_tensor, np.int32)
    per_core = []
    for c in range(CORES):
        sl = ci[c * NODES_PER_CORE:(c + 1) * NODES_PER_CORE]
        per_core.append(_plan_core(sl))

    # merge caps across cores per (chunk, window)
    caps = []
    for k in range(NCHUNK):
        caps.append([max(per_core[c][1][k]["caps"][w] for c in range(CORES))
                     for w in range(3)])

    # build wrapped idx arrays padded to merged caps
    idx_blobs = []
    orders = []
    for c in range(CORES):
        order, chunks = per_core[c]
        orders.append(order)
        cols = []
        for k in range(NCHUNK):
            for w in range(3):
                cap = caps[k][w]
                if cap == 0:
                    continue
                neu = (NEUTRAL_V1, NEUTRAL_V2, NEUTRAL_V3)[w]
                v = chunks[k]["views"][w]
                if v is None:
                    v = np.full((CH, cap), neu, np.int32)
                elif v.shape[1] < cap:
                    padw = np.full((CH, cap - v.shape[1]), neu, np.int32)
                    v = np.concatenate([v, padw], axis=1)
                cols.append(_wrap_idx(v, cap))
        idx_blobs.append(np.concatenate(cols, axis=1))
    total_cols = idx_blobs[0].shape[1]
    return caps, total_cols, idx_blobs, orders


_CACHE = {}


def kernel(input_tensor, check_index_tensor):
    from concourse.bass_utils import run_bass_kernel_spmd

    x = np.asarray(input_tensor, np.float32)
    ci = np.asarray(check_index_tensor, np.int32)

    key = (ci.shape, int(ci[::997, ::7].sum()), int(ci[1::1009, 3::5].sum()))
    if key not in _CACHE:
        caps, total_cols, idx_blobs, orders = _prepare(ci)
        nc = _build_bass((caps, total_cols))
        _CACHE.clear()
        _CACHE[key] = (nc, idx_blobs, orders)
    nc, idx_blobs, orders = _CACHE[key]

    table = pack_table(x)
    in_maps = [{"table": table, "idx": idx_blobs[c]} for c in range(CORES)]
    res = run_bass_kernel_spmd(nc, in_maps, list(range(CORES)))

    full = np.empty((NODES, B), np.float32)
    for c in range(CORES):
        part = res.results[c]["out"].view(np.float32)   # rows [ci, p, jb]
        part = (part.reshape(NCHUNK, 128, NB, B)
                .transpose(0, 2, 1, 3)
                .reshape(NODES_PER_CORE, B))            # -> chunk-q order
        base = c * NODES_PER_CORE
        full[base + orders[c]] = part
    return np.ascontiguousarray(full.T)                 # (B, N) f32
